# revision 8
# baseline (speedup 1.0000x reference)
"""Trainium2 Bass kernel for NeuralSDEGenerator sampling (Euler-Maruyama scan).

Math (per batch element b, all fp32):
  x0 = tanh(noise @ iw1 + ib1) @ iw2 + ib2
  for t in 0..T-1:
    f  = mlp3_drift([t_val, x])          # (1+8)->128->128->8
    g  = mlp3_diff ([t_val, x])          # (1+8)->128->128->64 -> [8,8]
    x += f*dt + einsum('dn,n->d', g, dW[t]*sqrt(dt))
  out[b] = concat over steps of x  -> [B, 100*8]

Mapping: pure data parallel over 8 cores (batch 65536 -> 8192/core).
On-chip layout is feature-major: activations [feature_partitions, batch_cols].
 - time row + layer-1 bias folded into per-step bias b1eff[t] = b1 + t*w1[0]
 - dt folded into drift_w3/b3; sqrt(dt) folded into the einsum-reduction matrix S
 - einsum: g in PSUM [64=(n*8+d), batch], dW broadcast-loaded [64, batch] via a
   replicated-read DMA from host-transposed dW, fused (g+b3)*dWb on DVE, then a
   0/1 matrix S [64,8] reduces over n on the PE, accumulating with f*dt.
 - path transposed to batch-major via DVE 32x32 block transposes, staged in an
   SBUF history buffer, flushed to DRAM every TF steps.
"""

import numpy as np
from contextlib import ExitStack

import concourse.bass as bass
import concourse.tile as tile
import concourse.mybir as mybir
from concourse import bacc
from concourse.bass_utils import run_bass_kernel_spmd

F32 = mybir.dt.float32
AF = mybir.ActivationFunctionType
OP = mybir.AluOpType

D = 8          # data dim
ND = 8         # noise dim
H = 128        # hidden
T_FULL = 99    # number of SDE steps
NCORES = 8
B_FULL = 65536
BC = B_FULL // NCORES   # 8192 batch per core
SC = 1024               # superchunk batch (free-dim width of one MLP pass)
NSC = BC // SC          # 8 superchunks
QW = 2048               # dwb load width (2 superchunks)
TF_DEF = 10             # history steps per output flush


def build_program(T=T_FULL, TF=TF_DEF):
    """Build the Bass/Tile program for one core (SPMD across 8 cores)."""
    O = T + 1  # output steps
    assert O % TF == 0
    nc = bacc.Bacc("TRN2", target_bir_lowering=False, debug=False)

    # ---- DRAM tensors (per-core shapes) ----
    dram = {}
    def din(name, shape):
        dram[name] = nc.dram_tensor(name, list(shape), F32, kind="ExternalInput")
        return dram[name]

    noiseT = din("noiseT", (ND, BC))
    dWT = din("dWT", (T, ND, BC))
    w14d = din("w14d", (128, 128))     # drift_w1[1:9] replicated at rows 32u..32u+8
    w14g = din("w14g", (128, 128))
    w2d = din("w2d", (H, H))
    w2g = din("w2g", (H, H))
    w3ddt = din("w3ddt", (H, D))       # drift_w3 * dt
    w3gp = din("w3gp", (H, D * ND))    # diff_w3, columns permuted to n*8+d
    smat = din("smat", (128, D))       # S[n*8+d, d] = sqrt(dt), replicated both halves
    b1effd = din("b1effd", (H, T))
    b1effg = din("b1effg", (H, T))
    b2d = din("b2d", (H, 1))
    b2g = din("b2g", (H, 1))
    b3gp = din("b3gp", (128, 1))       # diff_b3 permuted n-major, replicated both halves
    b3ddt = din("b3ddt", (128, 1))     # drift_b3 * dt, replicated at rows 32u..32u+8
    iw1 = din("iw1", (ND, H))
    ib1 = din("ib1", (H, 1))
    iw2 = din("iw2", (H, D))
    ib2 = din("ib2", (128, 1))         # init_b2 replicated at rows 32u..32u+8

    out = nc.dram_tensor("out", [BC, O * D], F32, kind="ExternalOutput")

    with tile.TileContext(nc) as tc, ExitStack() as ctx:
        cp = ctx.enter_context(tc.tile_pool(name="consts", bufs=1))
        xp = ctx.enter_context(tc.tile_pool(name="xstate", bufs=1))
        hp = ctx.enter_context(tc.tile_pool(name="acts", bufs=2))
        pp = ctx.enter_context(tc.tile_pool(name="prod", bufs=2))
        trp = ctx.enter_context(tc.tile_pool(name="trans", bufs=2))
        dwp = ctx.enter_context(tc.tile_pool(name="dwb", bufs=2))
        hip = ctx.enter_context(tc.tile_pool(name="hist", bufs=2))
        np_ = ctx.enter_context(tc.tile_pool(name="noise", bufs=2))
        pA = ctx.enter_context(tc.tile_pool(name="psA", bufs=1, space="PSUM"))
        pB = ctx.enter_context(tc.tile_pool(name="psB", bufs=1, space="PSUM"))
        pGD = ctx.enter_context(tc.tile_pool(name="psGD", bufs=2, space="PSUM"))

        # ---- load constants into SBUF ----
        def csb(name, shape):
            t = cp.tile(list(shape), F32, tag=name, name=f"sb_{name}")
            nc.sync.dma_start(t[...], dram[name].ap())
            return t

        W14d = csb("w14d", (128, 128))
        W14g = csb("w14g", (128, 128))
        W2d = csb("w2d", (H, H))
        W2g = csb("w2g", (H, H))
        W3ddt = csb("w3ddt", (H, D))
        W3gp = csb("w3gp", (H, D * ND))
        Smat = csb("smat", (128, D))
        B1effd = csb("b1effd", (H, T))
        B1effg = csb("b1effg", (H, T))
        B2d = csb("b2d", (H, 1))
        B2g = csb("b2g", (H, 1))
        B3gp = csb("b3gp", (128, 1))
        B3ddt = csb("b3ddt", (128, 1))
        IW1 = csb("iw1", (ND, H))
        IB1 = csb("ib1", (H, 1))
        IW2 = csb("iw2", (H, D))
        IB2 = csb("ib2", (128, 1))

        # ---- persistent x state: two tiles, superchunk s -> (q=s//4, rows 32*(s%4)+0..8)
        xq = []
        for q in range(2):
            t = xp.tile([128, SC], F32, tag=f"xq{q}", name=f"xq{q}")
            nc.vector.memset(t[...], 0.0)
            xq.append(t)

        hist = None

        def path_store(o):
            """Transpose x into batch-major blocks and stage into hist; flush on block end."""
            nonlocal hist
            tt = o % TF
            if tt == 0:
                hist = hip.tile([128, 2 * 32 * TF * D], F32, tag="hist", name="hist")
            for q in range(2):
                tr = trp.tile([128, SC], F32, tag="tr", name="tr")
                nc.vector.transpose(tr[...], xq[q][...])
                # tr[32*u + jl, 32*cb + d] = x(s=4q+u, d, j=32*cb+jl), d<8 valid
                src = tr.rearrange("p (cb dd) -> p cb dd", cb=32, dd=32)[:, :, 0:D]
                dst = hist.rearrange("p (qq cb tt d) -> p qq cb tt d",
                                     qq=2, cb=32, tt=TF, d=D)[:, q, :, tt, :]
                nc.vector.tensor_copy(dst, src)
            if tt == TF - 1:
                o0 = o - TF + 1
                # out[b, (o0..o0+TF)*8+d]; b = (4q+u)*1024 + cb*32 + jl
                ov = out.ap().rearrange("(qq u cb jl) w -> qq u cb jl w",
                                        qq=2, u=4, cb=32, jl=32)
                for q in range(2):
                    for u in range(4):
                        dv = ov[q, u, :, :, o0 * D:(o0 + TF) * D].transpose([1, 0, 2])
                        sv = hist[32 * u:32 * (u + 1), :].rearrange(
                            "p (qq cb ttd) -> p qq cb ttd", qq=2, cb=32, ttd=TF * D)[:, q, :, :]
                        nc.sync.dma_start(dv, sv)

        # ---- x0 = init MLP(noise) ----
        for s in range(NSC):
            q, u = divmod(s, 4)
            noi = np_.tile([ND, SC], F32, tag="noi", name="noi")
            nc.sync.dma_start(noi[...], noiseT.ap()[:, s * SC:(s + 1) * SC])
            psI = pA.tile([128, SC], F32, tag="L1", name="psI")
            for h in range(SC // 512):
                nc.tensor.matmul(psI[:, h * 512:(h + 1) * 512], IW1[...],
                                 noi[:, h * 512:(h + 1) * 512], start=True, stop=True)
            hI = hp.tile([128, SC], F32, tag="h1", name="hI")
            nc.scalar.activation(hI[...], psI[...], AF.Tanh, bias=IB1[:, 0:1])
            gd = pGD.tile([128, SC], F32, tag="GD", name="gd0")
            dxb = 32 * u
            for h in range(SC // 512):
                nc.tensor.matmul(gd[dxb:dxb + D, h * 512:(h + 1) * 512], IW2[...],
                                 hI[:, h * 512:(h + 1) * 512], start=True, stop=True,
                                 tile_position=(0, dxb))
            nc.vector.tensor_scalar(xq[q][dxb:dxb + D, :], gd[dxb:dxb + D, :],
                                    IB2[dxb:dxb + D, 0:1], None, op0=OP.add)
        path_store(0)

        # ---- main time loop ----
        for t in range(T):
            for q4 in range(BC // QW):
                # superchunks in this quarter share u-range: q4 even -> u in {0,1}
                # (dx rows 0-7/32-39, so g/prod/dwb live at rows 64-127);
                # q4 odd -> u in {2,3} (dx rows 64-71/96-103, g half at rows 0-63)
                gb = 64 if q4 % 2 == 0 else 0
                dwb = dwp.tile([128, QW], F32, tag="dwb", name="dwb")
                src = dWT.ap()[t][:, q4 * QW:(q4 + 1) * QW].unsqueeze(1).to_broadcast(
                    [ND, D, QW])
                nc.sync.dma_start(dwb[gb:gb + D * ND, :], src)
                for s in range(q4 * (QW // SC), (q4 + 1) * (QW // SC)):
                    q, u = divmod(s, 4)
                    xs = xq[q][32 * u:32 * u + D, :]
                    soff = (s % (QW // SC)) * SC

                    # drift MLP
                    ps1 = pA.tile([128, SC], F32, tag="L1", name="ps1")
                    for h in range(SC // 512):
                        sl = slice(h * 512, (h + 1) * 512)
                        nc.tensor.matmul(ps1[:, sl], W14d[32 * u:32 * u + D, :],
                                         xs[:, sl], start=True, stop=True,
                                         tile_position=(32 * u, 0))
                    h1 = hp.tile([128, SC], F32, tag="h1", name="h1")
                    nc.scalar.activation(h1[...], ps1[...], AF.Tanh,
                                         bias=B1effd[:, t:t + 1])
                    ps2 = pB.tile([128, SC], F32, tag="L2", name="ps2")
                    for h in range(SC // 512):
                        sl = slice(h * 512, (h + 1) * 512)
                        nc.tensor.matmul(ps2[:, sl], W2d[...], h1[:, sl],
                                         start=True, stop=True)
                    h2d = hp.tile([128, SC], F32, tag="h2d", name="h2d")
                    nc.scalar.activation(h2d[...], ps2[...], AF.Tanh, bias=B2d[:, 0:1])

                    # diffusion MLP
                    ps1g = pA.tile([128, SC], F32, tag="L1", name="ps1g")
                    for h in range(SC // 512):
                        sl = slice(h * 512, (h + 1) * 512)
                        nc.tensor.matmul(ps1g[:, sl], W14g[32 * u:32 * u + D, :],
                                         xs[:, sl], start=True, stop=True,
                                         tile_position=(32 * u, 0))
                    h1g = hp.tile([128, SC], F32, tag="h1", name="h1g")
                    nc.scalar.activation(h1g[...], ps1g[...], AF.Tanh,
                                         bias=B1effg[:, t:t + 1])
                    ps2g = pB.tile([128, SC], F32, tag="L2", name="ps2g")
                    for h in range(SC // 512):
                        sl = slice(h * 512, (h + 1) * 512)
                        nc.tensor.matmul(ps2g[:, sl], W2g[...], h1g[:, sl],
                                         start=True, stop=True)
                    h2g = hp.tile([128, SC], F32, tag="h2g", name="h2g")
                    nc.scalar.activation(h2g[...], ps2g[...], AF.Tanh, bias=B2g[:, 0:1])

                    # heads: g -> gd[gb:gb+64], dx -> gd[32u:32u+8]
                    dxb = 32 * u
                    gd = pGD.tile([128, SC], F32, tag="GD", name="gd")
                    for h in range(SC // 512):
                        sl = slice(h * 512, (h + 1) * 512)
                        nc.tensor.matmul(gd[gb:gb + D * ND, sl], W3gp[...], h2g[:, sl],
                                         start=True, stop=True, tile_position=(0, gb))
                        nc.tensor.matmul(gd[dxb:dxb + D, sl], W3ddt[...], h2d[:, sl],
                                         start=True, stop=False, tile_position=(0, dxb),
                                         skip_group_check=True)
                    prod = pp.tile([128, SC], F32, tag="prod", name="prod")
                    nc.vector.scalar_tensor_tensor(
                        prod[gb:gb + D * ND, :], gd[gb:gb + D * ND, :],
                        B3gp[gb:gb + D * ND, 0:1],
                        dwb[gb:gb + D * ND, soff:soff + SC], op0=OP.add, op1=OP.mult)
                    for h in range(SC // 512):
                        sl = slice(h * 512, (h + 1) * 512)
                        nc.tensor.matmul(gd[dxb:dxb + D, sl], Smat[gb:gb + D * ND, :],
                                         prod[gb:gb + D * ND, sl],
                                         start=False, stop=True, tile_position=(gb, dxb),
                                         skip_group_check=True)
                    # x += dx + b3d*dt
                    nc.vector.scalar_tensor_tensor(
                        xs, gd[dxb:dxb + D, :], B3ddt[dxb:dxb + D, 0:1], xs,
                        op0=OP.add, op1=OP.add)
            path_store(t + 1)

    nc.compile()
    return nc


_PROG = {}


def _get_prog(T=T_FULL, TF=TF_DEF):
    key = (T, TF)
    if key not in _PROG:
        _PROG[key] = build_program(T, TF)
    return _PROG[key]


def host_prep(inputs, T=T_FULL):
    """Host-side constant preparation + per-core input sharding."""
    f32 = np.float32
    dt = 1.0 / T
    sqdt = np.sqrt(np.float32(dt))
    t_vals = (np.arange(T) / T).astype(f32)

    def rep4(w):  # [8,128] -> [128,128] replicated at rows 32u..32u+8
        r = np.zeros((128, 128), f32)
        for u in range(4):
            r[32 * u:32 * u + w.shape[0], :] = w
        return r

    drift_w1 = np.asarray(inputs["drift_w1"], f32)
    diff_w1 = np.asarray(inputs["diff_w1"], f32)
    diff_w3 = np.asarray(inputs["diff_w3"], f32)
    consts = {
        "w14d": rep4(drift_w1[1:]),
        "w14g": rep4(diff_w1[1:]),
        "w2d": np.asarray(inputs["drift_w2"], f32),
        "w2g": np.asarray(inputs["diff_w2"], f32),
        "w3ddt": (np.asarray(inputs["drift_w3"], f32) * dt).astype(f32),
        "w3gp": diff_w3.reshape(H, D, ND).transpose(0, 2, 1).reshape(H, D * ND).copy(),
        "b1effd": (np.asarray(inputs["drift_b1"], f32)[None, :]
                   + t_vals[:, None] * drift_w1[0][None, :]).T.copy(),
        "b1effg": (np.asarray(inputs["diff_b1"], f32)[None, :]
                   + t_vals[:, None] * diff_w1[0][None, :]).T.copy(),
        "b2d": np.asarray(inputs["drift_b2"], f32).reshape(H, 1),
        "b2g": np.asarray(inputs["diff_b2"], f32).reshape(H, 1),
        "b3gp": np.tile(
            np.asarray(inputs["diff_b3"], f32).reshape(D, ND).T.reshape(D * ND, 1),
            (2, 1)),
        "b3ddt": rep4((np.asarray(inputs["drift_b3"], f32) * dt).reshape(D, 1).astype(f32))[:, :1],
        "iw1": np.asarray(inputs["init_w1"], f32),
        "ib1": np.asarray(inputs["init_b1"], f32).reshape(H, 1),
        "iw2": np.asarray(inputs["init_w2"], f32),
        "ib2": rep4(np.asarray(inputs["init_b2"], f32).reshape(D, 1))[:, :1],
    }
    smat = np.zeros((D * ND, D), f32)
    for n in range(ND):
        for d in range(D):
            smat[n * D + d, d] = sqdt
    consts["smat"] = np.tile(smat, (2, 1))
    consts = {k: np.ascontiguousarray(v, f32) for k, v in consts.items()}

    noise = np.asarray(inputs["initial_noise"], f32)
    dW = np.asarray(inputs["dW"], f32)[:T]
    in_maps = []
    for c in range(NCORES):
        sl = slice(c * BC, (c + 1) * BC)
        m = dict(consts)
        m["noiseT"] = np.ascontiguousarray(noise[sl].T)
        m["dWT"] = np.ascontiguousarray(dW[:, sl, :].transpose(0, 2, 1))
        in_maps.append(m)
    return in_maps


def kernel(**inputs):
    nc = _get_prog()
    in_maps = host_prep(inputs)
    res = run_bass_kernel_spmd(nc, in_maps, core_ids=list(range(NCORES)))
    return np.concatenate([r["out"] for r in res.results], axis=0)
